# revision 1
# baseline (speedup 1.0000x reference)
"""CompressedSparseAttention Trainium2 kernel (8 NeuronCores).

Sharding: data-parallel over batch (2) x tensor-parallel over head-pairs (4).
Core c handles batch b = c//4 and heads (2g, 2g+1) with g = c%4.
Each core computes its partial output  attn_out[:, hslice] @ wo[:, hslice].T
([2048, 512]); the host sums the 4 partials per batch (the tensor-parallel
all-reduce done on gather).

Layouts inside a core (SBUF partition dim first):
  xT        [512, 2048]   x[b].T, 4 chunks of [128, 2048], fp32r
  qT/kT     [128, 2048]   rows = 2 heads x 64 dims, bf16 after RoPE
  k_cT      [128, 511]    compressed keys (dims on partitions)
  v_aug     16 x [128, 130]  v chunks transposed to [pos, dim] + ones cols
  vc_aug    4 x [128, 130]   v_c chunks transposed to [w, dim] + ones cols
  scores^T  [keys<=128, q]   PSUM; exp'd on ACT; masks via gpsimd affine_select
  av^T      [65, 512]     PSUM per (head, q-block): rows 0-63 = sum exp*v,
                          row 64 = sum exp (denominator via ones column)
"""

import math

import numpy as np

import concourse.bass as bass
import concourse.mybir as mybir
import concourse.tile as tile
from concourse import bacc
from concourse.bass import ds
from concourse.masks import make_identity

B = 2
L = 2048
D = 512
H = 8
HD = 64
RATIO = 8
STRIDE = 4
WINDOW = 128
THETA = 10000.0
LC = (L - RATIO) // STRIDE + 1  # 511
NCORES = 8
NB = L // 512  # 4 q-blocks of 512
NCH = L // 128  # 16 q-chunks of 128
KD = D // 128  # 4 contraction chunks

F32 = mybir.dt.float32
F32R = mybir.dt.float32r
BF16 = mybir.dt.bfloat16
AF = mybir.ActivationFunctionType
ALU = mybir.AluOpType

_CACHE = {}


def _build_nc():
    nc = bacc.Bacc(
        "TRN2",
        target_bir_lowering=False,
        debug=False,
        num_devices=NCORES,
        name="csa",
    )

    # DRAM I/O (per-core views; float32r is bit-identical to float32)
    xT_d = nc.dram_tensor("xT", [D, L], F32R, kind="ExternalInput")
    wqT_d = nc.dram_tensor("wqT", [D, 128], F32R, kind="ExternalInput")
    wkT_d = nc.dram_tensor("wkT", [D, 128], F32R, kind="ExternalInput")
    wvT_d = nc.dram_tensor("wvT", [D, 128], F32R, kind="ExternalInput")
    wkcT_d = nc.dram_tensor("wkcT", [D, 128], F32R, kind="ExternalInput")
    wvcT_d = nc.dram_tensor("wvcT", [D, 128], F32R, kind="ExternalInput")
    woT_d = nc.dram_tensor("woT", [128, D], F32, kind="ExternalInput")
    cosT_d = nc.dram_tensor("cosT", [128, L], F32, kind="ExternalInput")
    sinST_d = nc.dram_tensor("sinST", [128, L], F32, kind="ExternalInput")
    gateb_d = nc.dram_tensor("gateb", [128, RATIO], F32, kind="ExternalInput")
    sink2_d = nc.dram_tensor("sink2", [1, 2], F32, kind="ExternalInput")
    outp_d = nc.dram_tensor("outp", [L, D], F32, kind="ExternalOutput")

    with tile.TileContext(nc) as tc:
        with tc.tile_pool(name="consts", bufs=1) as cp, \
             tc.tile_pool(name="work", bufs=1) as wp, \
             tc.tile_pool(name="ps", bufs=7, space="PSUM") as pp, \
             tc.tile_pool(name="pss", bufs=1, space="PSUM") as pps:

            # ---------------- init: DMA constants ----------------
            xT = []
            for c in range(KD):
                xt = cp.tile([128, L], F32R, tag=f"xt{c}", name=f"xt{c}")
                nc.sync.dma_start(out=xt, in_=xT_d[ds(128 * c, 128), :])
                xT.append(xt)

            def load_w(dram, tag):
                w = []
                for c in range(KD):
                    t = cp.tile([128, 128], F32R, tag=f"{tag}{c}", name=f"{tag}{c}")
                    nc.sync.dma_start(out=t, in_=dram[ds(128 * c, 128), :])
                    w.append(t)
                return w

            wq = load_w(wqT_d, "wq")
            wk = load_w(wkT_d, "wk")
            wv = load_w(wvT_d, "wv")
            wkc = load_w(wkcT_d, "wkc")
            wvc = load_w(wvcT_d, "wvc")

            woT = cp.tile([128, D], F32, tag="woT")
            nc.sync.dma_start(out=woT, in_=woT_d[:, :])
            woT_bf = cp.tile([128, D], BF16, tag="woT_bf")
            nc.scalar.copy(out=woT_bf, in_=woT)

            cosT = cp.tile([128, L], F32, tag="cosT")
            nc.sync.dma_start(out=cosT, in_=cosT_d[:, :])
            sinST = cp.tile([128, L], F32, tag="sinST")
            nc.sync.dma_start(out=sinST, in_=sinST_d[:, :])
            gateb = cp.tile([128, RATIO], F32, tag="gateb")
            nc.sync.dma_start(out=gateb, in_=gateb_d[:, :])
            sink2 = cp.tile([1, 2], F32, tag="sink2")
            nc.sync.dma_start(out=sink2, in_=sink2_d[:, :])

            # exp(sink) broadcast to all partitions
            exps = cp.tile([1, 2], F32, tag="exps")
            nc.scalar.activation(out=exps, in_=sink2, func=AF.Exp)
            expsb = cp.tile([128, 2], F32, tag="expsb")
            nc.gpsimd.partition_broadcast(expsb, exps)

            # identities for PE transpose
            ident_bf = cp.tile([128, 128], BF16, tag="ident_bf")
            make_identity(nc, ident_bf)
            ident_f = cp.tile([128, 128], F32, tag="ident_f")
            make_identity(nc, ident_f)

            # ---------------- P1: projections + RoPE + pooling ----------------
            qT = cp.tile([128, L], BF16, tag="qT")
            kT = cp.tile([128, L], BF16, tag="kT")
            vT_bf = cp.tile([128, L], BF16, tag="vT_bf")
            y_kc = cp.tile([128, L], F32, tag="y_kc")
            y_vc = cp.tile([128, L], F32, tag="y_vc")

            def project(wlist, qb):
                ps = pp.tile([128, 512], F32, tag="bank", name="proj_ps")
                for c in range(KD):
                    nc.tensor.matmul(
                        ps,
                        wlist[c],
                        xT[c][:, ds(512 * qb, 512)],
                        start=(c == 0),
                        stop=(c == KD - 1),
                    )
                return ps

            def rope_block(ps, outT, qb):
                qraw = wp.tile([128, 512], F32, tag="qraw", bufs=2, name="qraw")
                nc.scalar.copy(out=qraw, in_=ps)
                qsw = wp.tile([128, 512], F32, tag="qsw", bufs=2, name="qsw")
                for a, bb in ((0, 32), (32, 0), (64, 96), (96, 64)):
                    nc.gpsimd.tensor_copy(
                        out=qsw[ds(a, 32), :], in_=qraw[ds(bb, 32), :]
                    )
                m1 = wp.tile([128, 512], F32, tag="m1", bufs=2, name="m1")
                nc.vector.tensor_mul(m1, ps, cosT[:, ds(512 * qb, 512)])
                m2 = wp.tile([128, 512], F32, tag="m2", bufs=2, name="m2")
                nc.vector.tensor_mul(m2, qsw, sinST[:, ds(512 * qb, 512)])
                nc.vector.tensor_add(outT[:, ds(512 * qb, 512)], m1, m2)

            for qb in range(NB):
                ps = project(wq, qb)
                rope_block(ps, qT, qb)
            for qb in range(NB):
                ps = project(wk, qb)
                rope_block(ps, kT, qb)
            for qb in range(NB):
                ps = project(wv, qb)
                nc.scalar.copy(out=vT_bf[:, ds(512 * qb, 512)], in_=ps)
            for qb in range(NB):
                ps = project(wkc, qb)
                nc.scalar.copy(out=y_kc[:, ds(512 * qb, 512)], in_=ps)
            for qb in range(NB):
                ps = project(wvc, qb)
                nc.scalar.copy(out=y_vc[:, ds(512 * qb, 512)], in_=ps)

            # pooling: kc/vc[dim, w] = sum_r gate[r] * y[dim, 4w + r]
            def pool(y, out_bf):
                y4 = y.rearrange("p (w r) -> p r w", r=STRIDE)
                acc = [
                    wp.tile([128, LC], F32, tag="poolA", bufs=1, name="poolA"),
                    wp.tile([128, LC], F32, tag="poolB", bufs=1, name="poolB"),
                ]
                nc.vector.tensor_scalar(
                    out=acc[0],
                    in0=y4[:, 0, 0:LC],
                    scalar1=gateb[:, 0:1],
                    scalar2=None,
                    op0=ALU.mult,
                )
                for r in range(1, RATIO):
                    dst = out_bf if r == RATIO - 1 else acc[r % 2]
                    nc.vector.scalar_tensor_tensor(
                        out=dst,
                        in0=y4[:, r % STRIDE, (r // STRIDE):(r // STRIDE) + LC],
                        scalar=gateb[:, ds(r, 1)],
                        in1=acc[(r - 1) % 2],
                        op0=ALU.mult,
                        op1=ALU.add,
                    )

            k_cT = cp.tile([128, LC], BF16, tag="k_cT")
            v_cT = cp.tile([128, LC], BF16, tag="v_cT")
            pool(y_kc, k_cT)
            pool(y_vc, v_cT)

            # transpose v -> v_aug chunks [pos, dim] (+ones col at 64 and 129)
            v_aug = []
            for ch in range(NCH):
                va = cp.tile([128, 130], BF16, tag=f"v_aug{ch}", name=f"v_aug{ch}")
                nc.gpsimd.memset(va, 1.0)
                tp = pps.tile([128, 128], BF16, tag="small", name="tr_ps")
                nc.tensor.transpose(tp, vT_bf[:, ds(128 * ch, 128)], ident_bf)
                nc.vector.tensor_copy(out=va[:, 0:64], in_=tp[:, 0:64])
                nc.vector.tensor_copy(out=va[:, 65:129], in_=tp[:, 64:128])
                v_aug.append(va)

            vc_aug = []
            for ch in range(4):
                wlen = min(128, LC - 128 * ch)  # 128,128,128,127
                va = cp.tile([128, 130], BF16, tag=f"vc_aug{ch}", name=f"vc_aug{ch}")
                nc.gpsimd.memset(va, 1.0)
                tp = pps.tile([128, 128], BF16, tag="small", name="trc_ps")
                nc.tensor.transpose(
                    tp[0:wlen, :], v_cT[:, ds(128 * ch, wlen)], ident_bf
                )
                nc.vector.tensor_copy(out=va[0:wlen, 0:64], in_=tp[0:wlen, 0:64])
                nc.vector.tensor_copy(out=va[0:wlen, 65:129], in_=tp[0:wlen, 64:128])
                vc_aug.append(va)

            # ---------------- P2: attention ----------------
            rec = [cp.tile([128, NCH], F32, tag=f"rec{h}", name=f"rec{h}") for h in range(2)]
            avT = []  # [128, 512] bf16 per q-block: rows 0-63 h0, 64-127 h1
            for qb in range(NB):
                at = cp.tile([128, 512], BF16, tag=f"avT{qb}", name=f"avT{qb}")
                avT.append(at)

            for qb in range(NB):
                for h in range(2):
                    hs = 64 * h
                    qs = qT[ds(hs, 64), ds(512 * qb, 512)]
                    av = pp.tile([65, 512], F32, tag="bank", name=f"av_{qb}_{h}")
                    first_av = [True]

                    def av_mm(lhsT, rhs, cols, stop=False):
                        nc.tensor.matmul(
                            av[:, cols] if cols is not None else av,
                            lhsT,
                            rhs,
                            start=first_av[0],
                            stop=stop,
                            skip_group_check=True,
                        )
                        first_av[0] = False

                    # --- compressed branch ---
                    for wc in range(qb + 1):
                        wlen = min(128, LC - 128 * wc)
                        sc = pp.tile([128, 512], F32, tag="bank", name="sc_ps")
                        nc.tensor.matmul(
                            sc[0:wlen, :],
                            k_cT[ds(hs, 64), ds(128 * wc, wlen)],
                            qs,
                            start=True,
                            stop=True,
                        )
                        ex = wp.tile([128, 512], BF16, tag="exc", bufs=3, name="exc")
                        nc.scalar.activation(
                            out=ex[0:wlen, :], in_=sc[0:wlen, :], func=AF.Exp,
                            scale=0.125,
                        )
                        if wc >= qb - 1:
                            # causal: keep q_rel >= 4*w_rel + 7 - 512*(qb - wc)
                            nc.gpsimd.affine_select(
                                out=ex[0:wlen, :],
                                in_=ex[0:wlen, :],
                                compare_op=ALU.is_ge,
                                fill=0.0,
                                base=-7 + 512 * (qb - wc),
                                pattern=[[1, 512]],
                                channel_multiplier=-4,
                            )
                        av_mm(
                            vc_aug[wc][0:wlen, ds(65 * h, 65)],
                            ex[0:wlen, :],
                            None,
                        )

                    # --- local window branch ---
                    for sub in range(4):
                        c = 4 * qb + sub
                        qcs = qT[ds(hs, 64), ds(128 * c, 128)]
                        wps = pp.tile([128, 256], F32, tag="bank", name="win_ps")
                        if c > 0:
                            nc.tensor.matmul(
                                wps[:, 0:128],
                                kT[ds(hs, 64), ds(128 * (c - 1), 128)],
                                qcs,
                                start=True,
                                stop=True,
                                skip_group_check=True,
                            )
                        nc.tensor.matmul(
                            wps[:, 128:256],
                            kT[ds(hs, 64), ds(128 * c, 128)],
                            qcs,
                            start=True,
                            stop=True,
                            skip_group_check=True,
                        )
                        exw = wp.tile([128, 256], BF16, tag="exw", bufs=3, name="exw")
                        lo = 0 if c > 0 else 128
                        nc.scalar.activation(
                            out=exw[:, lo:256], in_=wps[:, lo:256], func=AF.Exp,
                            scale=0.125,
                        )
                        if c > 0:
                            # prev chunk: keep k_rel > q_rel
                            nc.gpsimd.affine_select(
                                out=exw[:, 0:128],
                                in_=exw[:, 0:128],
                                compare_op=ALU.is_gt,
                                fill=0.0,
                                base=0,
                                pattern=[[-1, 128]],
                                channel_multiplier=1,
                            )
                        # current chunk: keep q_rel >= k_rel
                        nc.gpsimd.affine_select(
                            out=exw[:, 128:256],
                            in_=exw[:, 128:256],
                            compare_op=ALU.is_ge,
                            fill=0.0,
                            base=0,
                            pattern=[[1, 128]],
                            channel_multiplier=-1,
                        )
                        cols = ds(128 * sub, 128)
                        if c > 0:
                            av_mm(
                                v_aug[c - 1][:, ds(65 * h, 65)], exw[:, 0:128], cols
                            )
                        av_mm(
                            v_aug[c][:, ds(65 * h, 65)], exw[:, 128:256], cols,
                            stop=(sub == 3),
                        )

                    # --- denominator -> reciprocal in [q, 1] layout ---
                    drow = wp.tile([1, 512], F32, tag="drow", bufs=2, name="drow")
                    nc.scalar.copy(out=drow, in_=av[64:65, :])
                    dcol = pps.tile([128, 4], F32, tag="small", name="dcol")
                    for c4 in range(4):
                        nc.tensor.transpose(
                            dcol[:, ds(c4, 1)],
                            drow[:, ds(128 * c4, 128)],
                            ident_f[0:1, 0:1],
                        )
                    dsb = wp.tile([128, 4], F32, tag="dsb", bufs=2, name="dsb")
                    nc.vector.tensor_scalar(
                        out=dsb, in0=dcol, scalar1=expsb[:, ds(h, 1)], scalar2=None,
                        op0=ALU.add,
                    )
                    nc.vector.reciprocal(
                        out=rec[h][:, ds(4 * qb, 4)], in_=dsb
                    )

                    # numerator rows -> SBUF (bf16) for the wo matmul
                    nc.scalar.copy(
                        out=avT[qb][ds(hs, 64), :], in_=av[0:64, :]
                    )

            # ---------------- P3: output projection + normalize ----------------
            for qb in range(NB):
                for sub in range(4):
                    c = 4 * qb + sub
                    wo0 = pp.tile([128, 512], F32, tag="bank", name="wo0")
                    nc.tensor.matmul(
                        wo0, avT[qb][0:64, ds(128 * sub, 128)], woT_bf[0:64, :],
                        start=True, stop=True,
                    )
                    wo1 = pp.tile([128, 512], F32, tag="bank", name="wo1")
                    nc.tensor.matmul(
                        wo1, avT[qb][64:128, ds(128 * sub, 128)], woT_bf[64:128, :],
                        start=True, stop=True,
                    )
                    t0 = wp.tile([128, 512], F32, tag="t0", bufs=2, name="t0")
                    nc.scalar.activation(
                        out=t0, in_=wo0, func=AF.Copy, scale=rec[0][:, ds(c, 1)]
                    )
                    osb = wp.tile([128, 512], F32, tag="osb", bufs=3, name="osb")
                    nc.vector.scalar_tensor_tensor(
                        out=osb,
                        in0=wo1,
                        scalar=rec[1][:, ds(c, 1)],
                        in1=t0,
                        op0=ALU.mult,
                        op1=ALU.add,
                    )
                    nc.sync.dma_start(out=outp_d[ds(128 * c, 128), :], in_=osb)

    nc.compile()
    return nc


def _host_prep(inputs):
    """Build the 8 per-core input maps from full inputs."""
    x = np.asarray(inputs["x"], dtype=np.float32)
    wq = np.asarray(inputs["wq"], dtype=np.float32)
    wk = np.asarray(inputs["wk"], dtype=np.float32)
    wv = np.asarray(inputs["wv"], dtype=np.float32)
    wo = np.asarray(inputs["wo"], dtype=np.float32)
    wk_c = np.asarray(inputs["wk_c"], dtype=np.float32)
    wv_c = np.asarray(inputs["wv_c"], dtype=np.float32)
    gate_logits = np.asarray(inputs["gate_logits"], dtype=np.float32)
    sink_logit = np.asarray(inputs["sink_logit"], dtype=np.float32)

    # rope tables
    half = HD // 2
    inv_freq = 1.0 / (THETA ** (np.arange(half, dtype=np.float32) / half))
    t = np.arange(L, dtype=np.float32)
    f = t[:, None] * inv_freq[None, :]  # [L, 32]
    cos32 = np.cos(f).T.astype(np.float32)  # [32, L]
    sin32 = np.sin(f).T.astype(np.float32)
    cosT = np.tile(cos32, (4, 1))  # rows: i%32
    sinST = np.concatenate([-sin32, sin32, -sin32, sin32], axis=0)

    g = np.exp(gate_logits - gate_logits.max())
    g = (g / g.sum()).astype(np.float32)
    gateb = np.broadcast_to(g[None, :], (128, RATIO)).copy()

    in_maps = []
    for core in range(NCORES):
        b, grp = divmod(core, 4)
        sl = slice(128 * grp, 128 * (grp + 1))
        in_maps.append(
            {
                "xT": np.ascontiguousarray(x[b].T),
                "wqT": np.ascontiguousarray(wq[sl, :].T),
                "wkT": np.ascontiguousarray(wk[sl, :].T),
                "wvT": np.ascontiguousarray(wv[sl, :].T),
                "wkcT": np.ascontiguousarray(wk_c[sl, :].T),
                "wvcT": np.ascontiguousarray(wv_c[sl, :].T),
                "woT": np.ascontiguousarray(wo[:, sl].T),
                "cosT": cosT,
                "sinST": sinST,
                "gateb": gateb,
                "sink2": np.ascontiguousarray(
                    sink_logit[2 * grp : 2 * grp + 2, 0][None, :]
                ),
            }
        )
    return in_maps


def kernel(**inputs) -> np.ndarray:
    from concourse.bass_utils import run_bass_kernel_spmd

    if "nc" not in _CACHE:
        _CACHE["nc"] = _build_nc()
    nc = _CACHE["nc"]

    in_maps = _host_prep(inputs)
    res = run_bass_kernel_spmd(nc, in_maps, core_ids=list(range(NCORES)))
    out = np.zeros((B, L, D), dtype=np.float32)
    for core in range(NCORES):
        b = core // 4
        out[b] += res.results[core]["outp"]
    return out



# revision 2
# speedup vs baseline: 1.0659x; 1.0659x over previous
"""CompressedSparseAttention Trainium2 kernel (8 NeuronCores).

Sharding: data-parallel over batch (2) x tensor-parallel over head-pairs (4).
Core c handles batch b = c//4 and heads (2g, 2g+1) with g = c%4.
Each core computes its partial output  attn_out[:, hslice] @ wo[:, hslice].T
([2048, 512] bf16); the host sums the 4 partials per batch in f32 (the
tensor-parallel all-reduce done on gather).

Per-dispatch I/O is minimized (it dominates the measured time):
  xbf   [512, 2048] bf16   x[b].T
  wpk   [512, 896]  bf16   packed wq|wqP|wk|wkP|wv|wkc|wvc column blocks,
                           where *P are row-permuted copies (RoPE half-swap
                           folded into the projection weights)
  wob   [128, 512]  bf16   wo[:, hslice].T
  misc  [1, 16]     f32    softmax(gate_logits), exp(sink) for the 2 heads
  outp  [2048, 512] bf16   output partial
cos/sin RoPE tables are Const tensors baked into the NEFF (zero per-dispatch
transfer cost).

Layouts inside a core (SBUF partition dim first):
  qT/kT     [128, 2048]  rows = 2 heads x 64 dims, bf16 after RoPE
  k_cT      [128, 511]   compressed keys (dims on partitions)
  v_aug     16 x [128, 130]  v chunks transposed to [pos, dim] + ones cols
  vc_aug    4 x [128, 130]   v_c chunks transposed to [w, dim] + ones cols
  scores^T  [keys<=128, q]   PSUM; exp'd on ACT; masks via gpsimd affine_select
  av^T      [65, 512]    PSUM per (head, q-block): rows 0-63 = sum exp*v,
                         row 64 = sum exp (denominator via ones column)
"""

import math

import numpy as np

import concourse.bass as bass
import concourse.mybir as mybir
import concourse.tile as tile
from concourse import bacc
from concourse.bass import ds
from concourse.masks import make_identity

B = 2
L = 2048
D = 512
H = 8
HD = 64
RATIO = 8
STRIDE = 4
WINDOW = 128
THETA = 10000.0
LC = (L - RATIO) // STRIDE + 1  # 511
NCORES = 8
NB = L // 512  # 4 q-blocks of 512
NCH = L // 128  # 16 q-chunks of 128
KD = D // 128  # 4 contraction chunks
NWC = 7  # packed weight column blocks: wq wqP wk wkP wv wkc wvc

F32 = mybir.dt.float32
BF16 = mybir.dt.bfloat16
AF = mybir.ActivationFunctionType
ALU = mybir.AluOpType

_CACHE = {}


def _rope_tables():
    half = HD // 2
    inv_freq = 1.0 / (THETA ** (np.arange(half, dtype=np.float32) / half))
    t = np.arange(L, dtype=np.float32)
    f = t[:, None] * inv_freq[None, :]  # [L, 32]
    cos32 = np.cos(f).T.astype(np.float32)  # [32, L]
    sin32 = np.sin(f).T.astype(np.float32)
    cosT = np.tile(cos32, (4, 1))  # rows: i%32
    sinST = np.concatenate([-sin32, sin32, -sin32, sin32], axis=0)
    return cosT, sinST


def _build_nc():
    nc = bacc.Bacc(
        "TRN2",
        target_bir_lowering=False,
        debug=False,
        num_devices=NCORES,
        name="csa",
    )

    # DRAM I/O (per-core views)
    xbf_d = nc.dram_tensor("xbf", [D, L], BF16, kind="ExternalInput")
    wpk_d = nc.dram_tensor("wpk", [D, 128 * NWC], BF16, kind="ExternalInput")
    wob_d = nc.dram_tensor("wob", [128, D], BF16, kind="ExternalInput")
    misc_d = nc.dram_tensor("misc", [1, 16], F32, kind="ExternalInput")
    outp_d = nc.dram_tensor("outp", [L, D], BF16, kind="ExternalOutput")

    cos_np, sinS_np = _rope_tables()
    cosT_d = nc.inline_tensor(cos_np, name="cosconst")
    sinST_d = nc.inline_tensor(sinS_np, name="sinconst")

    with tile.TileContext(nc) as tc:
        with tc.tile_pool(name="consts", bufs=1) as cp, \
             tc.tile_pool(name="work", bufs=1) as wp, \
             tc.tile_pool(name="ps", bufs=7, space="PSUM") as pp, \
             tc.tile_pool(name="pss", bufs=1, space="PSUM") as pps:

            # ---------------- init: DMA inputs + consts ----------------
            xT = []
            for c in range(KD):
                xt = cp.tile([128, L], BF16, tag=f"xt{c}", name=f"xt{c}")
                nc.sync.dma_start(out=xt, in_=xbf_d[ds(128 * c, 128), :])
                xT.append(xt)

            wpk = []
            for c in range(KD):
                t = cp.tile([128, 128 * NWC], BF16, tag=f"wpk{c}", name=f"wpk{c}")
                nc.sync.dma_start(out=t, in_=wpk_d[ds(128 * c, 128), :])
                wpk.append(t)

            wob = cp.tile([128, D], BF16, tag="wob")
            nc.sync.dma_start(out=wob, in_=wob_d[:, :])

            cosT = cp.tile([128, L], F32, tag="cosT")
            nc.sync.dma_start(out=cosT, in_=cosT_d[:, :])
            sinST = cp.tile([128, L], F32, tag="sinST")
            nc.sync.dma_start(out=sinST, in_=sinST_d[:, :])

            misc = cp.tile([1, 16], F32, tag="misc")
            nc.sync.dma_start(out=misc, in_=misc_d[:, :])
            gateb = cp.tile([128, RATIO], F32, tag="gateb")
            nc.gpsimd.partition_broadcast(gateb, misc[0:1, 0:RATIO])
            expsb = cp.tile([128, 2], F32, tag="expsb")
            nc.gpsimd.partition_broadcast(expsb, misc[0:1, RATIO:RATIO + 2])

            # identities for PE transpose
            ident_bf = cp.tile([128, 128], BF16, tag="ident_bf")
            make_identity(nc, ident_bf)
            ident_f = cp.tile([128, 128], F32, tag="ident_f")
            make_identity(nc, ident_f)

            # ---------------- P1: projections + RoPE + pooling ----------------
            qT = cp.tile([128, L], BF16, tag="qT")
            kT = cp.tile([128, L], BF16, tag="kT")
            vT_bf = cp.tile([128, L], BF16, tag="vT_bf")
            y_kc = cp.tile([128, L], F32, tag="y_kc")
            y_vc = cp.tile([128, L], F32, tag="y_vc")

            def project(wcol, qb, name):
                ps = pp.tile([128, 512], F32, tag="bank", name=name)
                for c in range(KD):
                    nc.tensor.matmul(
                        ps,
                        wpk[c][:, ds(128 * wcol, 128)],
                        xT[c][:, ds(512 * qb, 512)],
                        start=(c == 0),
                        stop=(c == KD - 1),
                    )
                return ps

            def rope_block(ps1, ps2, outT, qb):
                m1 = wp.tile([128, 512], F32, tag="m1", bufs=2, name="m1")
                nc.vector.tensor_mul(m1, ps1, cosT[:, ds(512 * qb, 512)])
                m2 = wp.tile([128, 512], F32, tag="m2", bufs=2, name="m2")
                nc.vector.tensor_mul(m2, ps2, sinST[:, ds(512 * qb, 512)])
                nc.vector.tensor_add(outT[:, ds(512 * qb, 512)], m1, m2)

            for qb in range(NB):
                ps1 = project(0, qb, "q1")
                ps2 = project(1, qb, "q2")
                rope_block(ps1, ps2, qT, qb)
            for qb in range(NB):
                ps1 = project(2, qb, "k1")
                ps2 = project(3, qb, "k2")
                rope_block(ps1, ps2, kT, qb)
            for qb in range(NB):
                ps = project(4, qb, "v")
                nc.scalar.copy(out=vT_bf[:, ds(512 * qb, 512)], in_=ps)
            for qb in range(NB):
                ps = project(5, qb, "kc")
                nc.scalar.copy(out=y_kc[:, ds(512 * qb, 512)], in_=ps)
            for qb in range(NB):
                ps = project(6, qb, "vc")
                nc.scalar.copy(out=y_vc[:, ds(512 * qb, 512)], in_=ps)

            # pooling: kc/vc[dim, w] = sum_r gate[r] * y[dim, 4w + r]
            def pool(y, out_bf):
                y4 = y.rearrange("p (w r) -> p r w", r=STRIDE)
                acc = [
                    wp.tile([128, LC], F32, tag="poolA", bufs=1, name="poolA"),
                    wp.tile([128, LC], F32, tag="poolB", bufs=1, name="poolB"),
                ]
                nc.vector.tensor_scalar(
                    out=acc[0],
                    in0=y4[:, 0, 0:LC],
                    scalar1=gateb[:, 0:1],
                    scalar2=None,
                    op0=ALU.mult,
                )
                for r in range(1, RATIO):
                    dst = out_bf if r == RATIO - 1 else acc[r % 2]
                    nc.vector.scalar_tensor_tensor(
                        out=dst,
                        in0=y4[:, r % STRIDE, (r // STRIDE):(r // STRIDE) + LC],
                        scalar=gateb[:, ds(r, 1)],
                        in1=acc[(r - 1) % 2],
                        op0=ALU.mult,
                        op1=ALU.add,
                    )

            k_cT = cp.tile([128, LC], BF16, tag="k_cT")
            v_cT = cp.tile([128, LC], BF16, tag="v_cT")
            pool(y_kc, k_cT)
            pool(y_vc, v_cT)

            # transpose v -> v_aug chunks [pos, dim] (+ones col at 64 and 129)
            v_aug = []
            for ch in range(NCH):
                va = cp.tile([128, 130], BF16, tag=f"v_aug{ch}", name=f"v_aug{ch}")
                nc.gpsimd.memset(va, 1.0)
                tp = pps.tile([128, 128], BF16, tag="small", name="tr_ps")
                nc.tensor.transpose(tp, vT_bf[:, ds(128 * ch, 128)], ident_bf)
                nc.vector.tensor_copy(out=va[:, 0:64], in_=tp[:, 0:64])
                nc.vector.tensor_copy(out=va[:, 65:129], in_=tp[:, 64:128])
                v_aug.append(va)

            vc_aug = []
            for ch in range(4):
                wlen = min(128, LC - 128 * ch)  # 128,128,128,127
                va = cp.tile([128, 130], BF16, tag=f"vc_aug{ch}", name=f"vc_aug{ch}")
                nc.gpsimd.memset(va, 1.0)
                tp = pps.tile([128, 128], BF16, tag="small", name="trc_ps")
                nc.tensor.transpose(
                    tp[0:wlen, :], v_cT[:, ds(128 * ch, wlen)], ident_bf
                )
                nc.vector.tensor_copy(out=va[0:wlen, 0:64], in_=tp[0:wlen, 0:64])
                nc.vector.tensor_copy(out=va[0:wlen, 65:129], in_=tp[0:wlen, 64:128])
                vc_aug.append(va)

            # ---------------- P2: attention ----------------
            rec = [cp.tile([128, NCH], F32, tag=f"rec{h}", name=f"rec{h}") for h in range(2)]
            avT = []  # [128, 512] bf16 per q-block: rows 0-63 h0, 64-127 h1
            for qb in range(NB):
                at = cp.tile([128, 512], BF16, tag=f"avT{qb}", name=f"avT{qb}")
                avT.append(at)

            for qb in range(NB):
                for h in range(2):
                    hs = 64 * h
                    qs = qT[ds(hs, 64), ds(512 * qb, 512)]
                    av = pp.tile([65, 512], F32, tag="bank", name=f"av_{qb}_{h}")
                    first_av = [True]

                    def av_mm(lhsT, rhs, cols, stop=False):
                        nc.tensor.matmul(
                            av[:, cols] if cols is not None else av,
                            lhsT,
                            rhs,
                            start=first_av[0],
                            stop=stop,
                            skip_group_check=True,
                        )
                        first_av[0] = False

                    # --- compressed branch ---
                    for wc in range(qb + 1):
                        wlen = min(128, LC - 128 * wc)
                        sc = pp.tile([128, 512], F32, tag="bank", name="sc_ps")
                        nc.tensor.matmul(
                            sc[0:wlen, :],
                            k_cT[ds(hs, 64), ds(128 * wc, wlen)],
                            qs,
                            start=True,
                            stop=True,
                        )
                        ex = wp.tile([128, 512], BF16, tag="exc", bufs=3, name="exc")
                        nc.scalar.activation(
                            out=ex[0:wlen, :], in_=sc[0:wlen, :], func=AF.Exp,
                            scale=0.125,
                        )
                        if wc >= qb - 1:
                            # causal: keep q_rel >= 4*w_rel + 7 - 512*(qb - wc)
                            nc.gpsimd.affine_select(
                                out=ex[0:wlen, :],
                                in_=ex[0:wlen, :],
                                compare_op=ALU.is_ge,
                                fill=0.0,
                                base=-7 + 512 * (qb - wc),
                                pattern=[[1, 512]],
                                channel_multiplier=-4,
                            )
                        av_mm(
                            vc_aug[wc][0:wlen, ds(65 * h, 65)],
                            ex[0:wlen, :],
                            None,
                        )

                    # --- local window branch ---
                    for sub in range(4):
                        c = 4 * qb + sub
                        qcs = qT[ds(hs, 64), ds(128 * c, 128)]
                        wps = pp.tile([128, 256], F32, tag="bank", name="win_ps")
                        if c > 0:
                            nc.tensor.matmul(
                                wps[:, 0:128],
                                kT[ds(hs, 64), ds(128 * (c - 1), 128)],
                                qcs,
                                start=True,
                                stop=True,
                                skip_group_check=True,
                            )
                        nc.tensor.matmul(
                            wps[:, 128:256],
                            kT[ds(hs, 64), ds(128 * c, 128)],
                            qcs,
                            start=True,
                            stop=True,
                            skip_group_check=True,
                        )
                        exw = wp.tile([128, 256], BF16, tag="exw", bufs=3, name="exw")
                        lo = 0 if c > 0 else 128
                        nc.scalar.activation(
                            out=exw[:, lo:256], in_=wps[:, lo:256], func=AF.Exp,
                            scale=0.125,
                        )
                        if c > 0:
                            # prev chunk: keep k_rel > q_rel
                            nc.gpsimd.affine_select(
                                out=exw[:, 0:128],
                                in_=exw[:, 0:128],
                                compare_op=ALU.is_gt,
                                fill=0.0,
                                base=0,
                                pattern=[[-1, 128]],
                                channel_multiplier=1,
                            )
                        # current chunk: keep q_rel >= k_rel
                        nc.gpsimd.affine_select(
                            out=exw[:, 128:256],
                            in_=exw[:, 128:256],
                            compare_op=ALU.is_ge,
                            fill=0.0,
                            base=0,
                            pattern=[[1, 128]],
                            channel_multiplier=-1,
                        )
                        cols = ds(128 * sub, 128)
                        if c > 0:
                            av_mm(
                                v_aug[c - 1][:, ds(65 * h, 65)], exw[:, 0:128], cols
                            )
                        av_mm(
                            v_aug[c][:, ds(65 * h, 65)], exw[:, 128:256], cols,
                            stop=(sub == 3),
                        )

                    # --- denominator -> reciprocal in [q, 1] layout ---
                    drow = wp.tile([1, 512], F32, tag="drow", bufs=2, name="drow")
                    nc.scalar.copy(out=drow, in_=av[64:65, :])
                    dcol = pps.tile([128, 4], F32, tag="small", name="dcol")
                    for c4 in range(4):
                        nc.tensor.transpose(
                            dcol[:, ds(c4, 1)],
                            drow[:, ds(128 * c4, 128)],
                            ident_f[0:1, 0:1],
                        )
                    dsb = wp.tile([128, 4], F32, tag="dsb", bufs=2, name="dsb")
                    nc.vector.tensor_scalar(
                        out=dsb, in0=dcol, scalar1=expsb[:, ds(h, 1)], scalar2=None,
                        op0=ALU.add,
                    )
                    nc.vector.reciprocal(
                        out=rec[h][:, ds(4 * qb, 4)], in_=dsb
                    )

                    # numerator rows -> SBUF (bf16) for the wo matmul
                    nc.scalar.copy(
                        out=avT[qb][ds(hs, 64), :], in_=av[0:64, :]
                    )

            # ---------------- P3: output projection + normalize ----------------
            for qb in range(NB):
                for sub in range(4):
                    c = 4 * qb + sub
                    wo0 = pp.tile([128, 512], F32, tag="bank", name="wo0")
                    nc.tensor.matmul(
                        wo0, avT[qb][0:64, ds(128 * sub, 128)], wob[0:64, :],
                        start=True, stop=True,
                    )
                    wo1 = pp.tile([128, 512], F32, tag="bank", name="wo1")
                    nc.tensor.matmul(
                        wo1, avT[qb][64:128, ds(128 * sub, 128)], wob[64:128, :],
                        start=True, stop=True,
                    )
                    t0 = wp.tile([128, 512], F32, tag="t0", bufs=2, name="t0")
                    nc.scalar.activation(
                        out=t0, in_=wo0, func=AF.Copy, scale=rec[0][:, ds(c, 1)]
                    )
                    osb = wp.tile([128, 512], BF16, tag="osb", bufs=3, name="osb")
                    nc.vector.scalar_tensor_tensor(
                        out=osb,
                        in0=wo1,
                        scalar=rec[1][:, ds(c, 1)],
                        in1=t0,
                        op0=ALU.mult,
                        op1=ALU.add,
                    )
                    nc.sync.dma_start(out=outp_d[ds(128 * c, 128), :], in_=osb)

    nc.compile()
    return nc


def _rope_perm():
    """Row permutation realizing the RoPE half-swap within each 64-dim head."""
    p = np.arange(128)
    base = (p // 64) * 64
    lr = p % 64
    return base + (lr + 32) % 64


def _host_prep(inputs):
    """Build the 8 per-core input maps from full inputs."""
    import ml_dtypes

    bf16 = ml_dtypes.bfloat16
    x = np.asarray(inputs["x"], dtype=np.float32)
    wq = np.asarray(inputs["wq"], dtype=np.float32)
    wk = np.asarray(inputs["wk"], dtype=np.float32)
    wv = np.asarray(inputs["wv"], dtype=np.float32)
    wo = np.asarray(inputs["wo"], dtype=np.float32)
    wk_c = np.asarray(inputs["wk_c"], dtype=np.float32)
    wv_c = np.asarray(inputs["wv_c"], dtype=np.float32)
    gate_logits = np.asarray(inputs["gate_logits"], dtype=np.float32)
    sink_logit = np.asarray(inputs["sink_logit"], dtype=np.float32)

    g = np.exp(gate_logits - gate_logits.max())
    g = (g / g.sum()).astype(np.float32)

    perm = _rope_perm()

    in_maps = []
    for core in range(NCORES):
        b, grp = divmod(core, 4)
        sl = slice(128 * grp, 128 * (grp + 1))
        wq_s = wq[sl, :]
        wk_s = wk[sl, :]
        # packed [D, 896]: columns wq | wqP | wk | wkP | wv | wkc | wvc
        wpk = np.concatenate(
            [
                wq_s.T,
                wq_s[perm, :].T,
                wk_s.T,
                wk_s[perm, :].T,
                wv[sl, :].T,
                wk_c[sl, :].T,
                wv_c[sl, :].T,
            ],
            axis=1,
        ).astype(bf16)
        misc = np.zeros((1, 16), np.float32)
        misc[0, 0:RATIO] = g
        misc[0, RATIO:RATIO + 2] = np.exp(sink_logit[2 * grp:2 * grp + 2, 0])
        in_maps.append(
            {
                "xbf": np.ascontiguousarray(x[b].T).astype(bf16),
                "wpk": np.ascontiguousarray(wpk),
                "wob": np.ascontiguousarray(wo[:, sl].T).astype(bf16),
                "misc": misc,
            }
        )
    return in_maps


def kernel(**inputs) -> np.ndarray:
    from concourse.bass_utils import run_bass_kernel_spmd

    if "nc" not in _CACHE:
        _CACHE["nc"] = _build_nc()
    nc = _CACHE["nc"]

    in_maps = _host_prep(inputs)
    res = run_bass_kernel_spmd(nc, in_maps, core_ids=list(range(NCORES)))
    out = np.zeros((B, L, D), dtype=np.float32)
    for core in range(NCORES):
        b = core // 4
        out[b] += res.results[core]["outp"].astype(np.float32)
    return out


# revision 11
# speedup vs baseline: 2.6953x; 2.5286x over previous
"""CompressedSparseAttention Trainium2 kernel — 1-core variant, v2.

Sharding: none — a single core processes both batches (loop) and all 8
heads (group loop). Minimizing per-dispatch I/O dominates the measured
time; the device program is additionally restructured for less work:
  - compressed KV pooling runs on x BEFORE projection (linearity), once
    per batch instead of once per head-group
  - window-branch exp + causal masks batched into [128, 512] ops
  - RoPE multiplies emit bf16
  - per-group scratch is double-buffered so consecutive groups overlap

Per-core inputs:
  xbf   [1024, 2048] bf16  x[0].T | x[1].T stacked
  wpk   [512, 3584] bf16   4 group-blocks of 896 cols:
                           wq|wqP|wk|wkP|wv|wkc|wvc for heads (2g, 2g+1),
                           where *P are row-permuted copies (RoPE half-swap
                           folded into the projection weights)
  wob   [512, 512]  bf16   wo.T (rows = head dims)
  misc  [1, 16]     f32    softmax(gate_logits) | exp(sink_logit[0..8))
  outp  [4096, 512] bf16   finished output, batches stacked
cos/sin RoPE tables are Const tensors baked into the NEFF.
"""

import math

import numpy as np

import concourse.bass as bass
import concourse.mybir as mybir
import concourse.tile as tile
from concourse import bacc
from concourse.bass import ds
from concourse.masks import make_identity

B = 2
L = 2048
D = 512
H = 8
HD = 64
RATIO = 8
STRIDE = 4
WINDOW = 128
THETA = 10000.0
LC = (L - RATIO) // STRIDE + 1  # 511
NCORES = 1
NGRP = 4  # head-pair groups
NB = L // 512  # 4 q-blocks of 512
NCH = L // 128  # 16 q-chunks of 128
KD = D // 128  # 4 contraction chunks
NWC = 7  # weight column blocks per group: wq wqP wk wkP wv wkc wvc

F32 = mybir.dt.float32
BF16 = mybir.dt.bfloat16
AF = mybir.ActivationFunctionType
ALU = mybir.AluOpType

_CACHE = {}


def _rope_tables():
    half = HD // 2
    inv_freq = 1.0 / (THETA ** (np.arange(half, dtype=np.float32) / half))
    t = np.arange(L, dtype=np.float32)
    f = t[:, None] * inv_freq[None, :]  # [L, 32]
    cos32 = np.cos(f).T.astype(np.float32)  # [32, L]
    sin32 = np.sin(f).T.astype(np.float32)
    cosT = np.tile(cos32, (4, 1))  # rows: i%32
    sinST = np.concatenate([-sin32, sin32, -sin32, sin32], axis=0)
    return cosT, sinST


def _build_nc():
    nc = bacc.Bacc(
        "TRN2",
        target_bir_lowering=False,
        debug=False,
        num_devices=NCORES,
        name="csa1b",
    )

    xbf_d = nc.dram_tensor("xbf", [B * D, L], BF16, kind="ExternalInput")
    wpk_d = nc.dram_tensor("wpk", [D, 128 * NWC * NGRP], BF16, kind="ExternalInput")
    wob_d = nc.dram_tensor("wob", [D, D], BF16, kind="ExternalInput")
    misc_d = nc.dram_tensor("misc", [1, 16], F32, kind="ExternalInput")
    outp_d = nc.dram_tensor("outp", [B * L, D], BF16, kind="ExternalOutput")

    cos_np, sinS_np = _rope_tables()
    cosT_d = nc.inline_tensor(cos_np, name="cosconst")
    sinST_d = nc.inline_tensor(sinS_np, name="sinconst")

    with tile.TileContext(nc) as tc:
        with tc.tile_pool(name="consts", bufs=1) as cp, \
             tc.tile_pool(name="work", bufs=1) as wp, \
             tc.tile_pool(name="ps", bufs=7, space="PSUM") as pp, \
             tc.tile_pool(name="pss", bufs=1, space="PSUM") as pps:

            # ---------------- init: DMA inputs + consts ----------------
            wpk = []
            for c in range(KD):
                t = cp.tile([128, 128 * NWC * NGRP], BF16, tag=f"wpk{c}",
                            name=f"wpk{c}")
                nc.sync.dma_start(out=t, in_=wpk_d[ds(128 * c, 128), :])
                wpk.append(t)

            wob_t = []
            for g in range(NGRP):
                t = cp.tile([128, D], BF16, tag=f"wob{g}", name=f"wob{g}")
                nc.sync.dma_start(out=t, in_=wob_d[ds(128 * g, 128), :])
                wob_t.append(t)

            cosT = cp.tile([128, L], F32, tag="cosT")
            nc.sync.dma_start(out=cosT, in_=cosT_d[:, :])
            sinST = cp.tile([128, L], F32, tag="sinST")
            nc.sync.dma_start(out=sinST, in_=sinST_d[:, :])

            misc = cp.tile([1, 16], F32, tag="misc")
            nc.sync.dma_start(out=misc, in_=misc_d[:, :])
            gateb = cp.tile([128, RATIO], F32, tag="gateb")
            nc.gpsimd.partition_broadcast(gateb, misc[0:1, 0:RATIO])
            expsb = cp.tile([128, H], F32, tag="expsb")
            nc.gpsimd.partition_broadcast(expsb, misc[0:1, RATIO:RATIO + H])

            ident_bf = cp.tile([128, 128], BF16, tag="ident_bf")
            make_identity(nc, ident_bf)
            ident_f = cp.tile([128, 128], F32, tag="ident_f")
            make_identity(nc, ident_f)

            # persistent per-(group, q-block) attention numerators + recips
            avT = [
                [cp.tile([128, 512], BF16, tag=f"avT{g}_{qb}", name=f"avT{g}_{qb}")
                 for qb in range(NB)]
                for g in range(NGRP)
            ]
            rec = [
                [cp.tile([128, NCH], F32, tag=f"rec{g}{h}", name=f"rec{g}{h}")
                 for h in range(2)]
                for g in range(NGRP)
            ]

            def project(xT, wcol, qb, name):
                ps = pp.tile([128, 512], F32, tag="bank", name=name)
                for c in range(KD):
                    nc.tensor.matmul(
                        ps,
                        wpk[c][:, ds(128 * wcol, 128)],
                        xT[c][:, ds(512 * qb, 512)],
                        start=(c == 0),
                        stop=(c == KD - 1),
                    )
                return ps

            def rope_block(ps1, ps2, outT, qb):
                m1 = wp.tile([128, 512], BF16, tag="m1", bufs=2, name="m1")
                nc.vector.tensor_mul(m1, ps1, cosT[:, ds(512 * qb, 512)])
                m2 = wp.tile([128, 512], BF16, tag="m2", bufs=2, name="m2")
                nc.vector.tensor_mul(m2, ps2, sinST[:, ds(512 * qb, 512)])
                nc.vector.tensor_add(outT[:, ds(512 * qb, 512)], m1, m2)

            def pool(y, out_bf):
                # out_bf[p, w] = sum_r gate[r] * y[p, 4w + r]
                y4 = y.rearrange("p (w r) -> p r w", r=STRIDE)
                acc = [
                    wp.tile([128, LC], F32, tag="poolA", bufs=2, name="poolA"),
                    wp.tile([128, LC], F32, tag="poolB", bufs=2, name="poolB"),
                ]
                nc.vector.tensor_scalar(
                    out=acc[0],
                    in0=y4[:, 0, 0:LC],
                    scalar1=gateb[:, 0:1],
                    scalar2=None,
                    op0=ALU.mult,
                )
                for r in range(1, RATIO):
                    dst = out_bf if r == RATIO - 1 else acc[r % 2]
                    nc.vector.scalar_tensor_tensor(
                        out=dst,
                        in0=y4[:, r % STRIDE, (r // STRIDE):(r // STRIDE) + LC],
                        scalar=gateb[:, ds(r, 1)],
                        in1=acc[(r - 1) % 2],
                        op0=ALU.mult,
                        op1=ALU.add,
                    )

            for b in range(B):
                # x chunks for this batch
                xT = []
                for c in range(KD):
                    xt = wp.tile([128, L], BF16, tag=f"xt{c}", bufs=2,
                                 name=f"xt{c}_{b}")
                    nc.sync.dma_start(
                        out=xt, in_=xbf_d[ds(512 * b + 128 * c, 128), :])
                    xT.append(xt)

                # pooled x (compressed tokens), shared by all groups
                x_cT = []
                for c in range(KD):
                    xc = wp.tile([128, LC], BF16, tag=f"xc{c}", bufs=2,
                                 name=f"xc{c}_{b}")
                    pool(xT[c], xc)
                    x_cT.append(xc)

                for g in range(NGRP):
                    wbase = NWC * g

                    # ---------- P1: projections + RoPE ----------
                    qT = wp.tile([128, L], BF16, tag="qT", bufs=2, name=f"qT{b}{g}")
                    kT = wp.tile([128, L], BF16, tag="kT", bufs=2, name=f"kT{b}{g}")
                    vT_bf = wp.tile([128, L], BF16, tag="vT", bufs=2,
                                    name=f"vT{b}{g}")
                    for qb in range(NB):
                        ps1 = project(xT, wbase + 0, qb, "q1")
                        ps2 = project(xT, wbase + 1, qb, "q2")
                        rope_block(ps1, ps2, qT, qb)
                    for qb in range(NB):
                        ps1 = project(xT, wbase + 2, qb, "k1")
                        ps2 = project(xT, wbase + 3, qb, "k2")
                        rope_block(ps1, ps2, kT, qb)
                    for qb in range(NB):
                        ps = project(xT, wbase + 4, qb, "v")
                        nc.scalar.copy(out=vT_bf[:, ds(512 * qb, 512)], in_=ps)

                    # compressed K/V: project pooled x (511 cols)
                    k_cT = wp.tile([128, LC], BF16, tag="k_cT", bufs=2,
                                   name=f"k_cT{b}{g}")
                    v_cT = wp.tile([128, LC], BF16, tag="v_cT", bufs=2,
                                   name=f"v_cT{b}{g}")
                    for wcol, dst in ((wbase + 5, k_cT), (wbase + 6, v_cT)):
                        ps = pp.tile([128, LC], F32, tag="bank", name="kcvc")
                        for c in range(KD):
                            nc.tensor.matmul(
                                ps,
                                wpk[c][:, ds(128 * wcol, 128)],
                                x_cT[c],
                                start=(c == 0),
                                stop=(c == KD - 1),
                            )
                        nc.scalar.copy(out=dst, in_=ps)

                    # transpose v -> [pos, dim] chunks with ones cols
                    v_aug = []
                    for ch in range(NCH):
                        va = wp.tile([128, 130], BF16, tag=f"v_aug{ch}", bufs=2,
                                     name=f"v_aug{ch}_{b}{g}")
                        nc.gpsimd.memset(va, 1.0)
                        tp = pps.tile([128, 128], BF16, tag="small", name="tr_ps")
                        nc.tensor.transpose(
                            tp, vT_bf[:, ds(128 * ch, 128)], ident_bf)
                        nc.vector.tensor_copy(out=va[:, 0:64], in_=tp[:, 0:64])
                        nc.vector.tensor_copy(out=va[:, 65:129], in_=tp[:, 64:128])
                        v_aug.append(va)

                    vc_aug = []
                    for ch in range(4):
                        wlen = min(128, LC - 128 * ch)  # 128,128,128,127
                        va = wp.tile([128, 130], BF16, tag=f"vc_aug{ch}", bufs=2,
                                     name=f"vc_aug{ch}_{b}{g}")
                        nc.gpsimd.memset(va, 1.0)
                        tp = pps.tile([128, 128], BF16, tag="small", name="trc_ps")
                        nc.tensor.transpose(
                            tp[0:wlen, :], v_cT[:, ds(128 * ch, wlen)], ident_bf
                        )
                        nc.vector.tensor_copy(
                            out=va[0:wlen, 0:64], in_=tp[0:wlen, 0:64])
                        nc.vector.tensor_copy(
                            out=va[0:wlen, 65:129], in_=tp[0:wlen, 64:128])
                        vc_aug.append(va)

                    # ---------- P2: attention ----------
                    for qb in range(NB):
                        for h in range(2):
                            hs = 64 * h
                            qs = qT[ds(hs, 64), ds(512 * qb, 512)]
                            av = pp.tile([65, 512], F32, tag="bank",
                                         name=f"av_{b}_{g}_{qb}_{h}")
                            first_av = [True]

                            def av_mm(lhsT, rhs, cols, stop=False):
                                nc.tensor.matmul(
                                    av[:, cols] if cols is not None else av,
                                    lhsT,
                                    rhs,
                                    start=first_av[0],
                                    stop=stop,
                                    skip_group_check=True,
                                )
                                first_av[0] = False

                            # --- compressed branch ---
                            for wc in range(qb + 1):
                                wlen = min(128, LC - 128 * wc)
                                sc = pp.tile([128, 512], F32, tag="bank",
                                             name="sc_ps")
                                nc.tensor.matmul(
                                    sc[0:wlen, :],
                                    k_cT[ds(hs, 64), ds(128 * wc, wlen)],
                                    qs,
                                    start=True,
                                    stop=True,
                                )
                                ex = wp.tile([128, 512], BF16, tag="exc", bufs=3,
                                             name="exc")
                                nc.scalar.activation(
                                    out=ex[0:wlen, :], in_=sc[0:wlen, :],
                                    func=AF.Exp, scale=0.125,
                                )
                                if wc >= qb - 1:
                                    # keep q_rel >= 4*w_rel + 7 - 512*(qb-wc)
                                    nc.gpsimd.affine_select(
                                        out=ex[0:wlen, :],
                                        in_=ex[0:wlen, :],
                                        compare_op=ALU.is_ge,
                                        fill=0.0,
                                        base=-7 + 512 * (qb - wc),
                                        pattern=[[1, 512]],
                                        channel_multiplier=-4,
                                    )
                                av_mm(
                                    vc_aug[wc][0:wlen, ds(65 * h, 65)],
                                    ex[0:wlen, :],
                                    None,
                                )

                            # --- local window branch, batched over sub ---
                            psP = pp.tile([128, 512], F32, tag="bank",
                                          name="winP")
                            psC = pp.tile([128, 512], F32, tag="bank",
                                          name="winC")
                            for sub in range(4):
                                c = 4 * qb + sub
                                qcs = qT[ds(hs, 64), ds(128 * c, 128)]
                                if c > 0:
                                    nc.tensor.matmul(
                                        psP[:, ds(128 * sub, 128)],
                                        kT[ds(hs, 64), ds(128 * (c - 1), 128)],
                                        qcs,
                                        start=True, stop=True,
                                        skip_group_check=True,
                                    )
                                nc.tensor.matmul(
                                    psC[:, ds(128 * sub, 128)],
                                    kT[ds(hs, 64), ds(128 * c, 128)],
                                    qcs,
                                    start=True, stop=True,
                                    skip_group_check=True,
                                )
                            exwP = wp.tile([128, 512], BF16, tag="exwP", bufs=3,
                                           name="exwP")
                            exwC = wp.tile([128, 512], BF16, tag="exwC", bufs=3,
                                           name="exwC")
                            lo = 128 if qb == 0 else 0
                            nsub = 3 if qb == 0 else 4
                            nc.scalar.activation(
                                out=exwP[:, lo:512], in_=psP[:, lo:512],
                                func=AF.Exp, scale=0.125,
                            )
                            nc.scalar.activation(
                                out=exwC, in_=psC, func=AF.Exp, scale=0.125,
                            )
                            # prev chunks: keep k_rel > q_rel
                            nc.gpsimd.affine_select(
                                out=exwP[:, lo:512],
                                in_=exwP[:, lo:512],
                                compare_op=ALU.is_gt,
                                fill=0.0,
                                base=0,
                                pattern=[[0, nsub], [-1, 128]],
                                channel_multiplier=1,
                            )
                            # current chunks: keep q_rel >= k_rel
                            nc.gpsimd.affine_select(
                                out=exwC,
                                in_=exwC,
                                compare_op=ALU.is_ge,
                                fill=0.0,
                                base=0,
                                pattern=[[0, 4], [1, 128]],
                                channel_multiplier=-1,
                            )
                            for sub in range(4):
                                c = 4 * qb + sub
                                cols = ds(128 * sub, 128)
                                if c > 0:
                                    av_mm(
                                        v_aug[c - 1][:, ds(65 * h, 65)],
                                        exwP[:, ds(128 * sub, 128)], cols
                                    )
                                av_mm(
                                    v_aug[c][:, ds(65 * h, 65)],
                                    exwC[:, ds(128 * sub, 128)],
                                    cols, stop=(sub == 3),
                                )

                            # --- denominator -> reciprocal in [q, 1] layout ---
                            drow = wp.tile([1, 512], F32, tag="drow", bufs=2,
                                           name="drow")
                            nc.scalar.copy(out=drow, in_=av[64:65, :])
                            dcol = pps.tile([128, 4], F32, tag="small",
                                            name="dcol")
                            for c4 in range(4):
                                nc.tensor.transpose(
                                    dcol[:, ds(c4, 1)],
                                    drow[:, ds(128 * c4, 128)],
                                    ident_f[0:1, 0:1],
                                )
                            dsb = wp.tile([128, 4], F32, tag="dsb", bufs=2,
                                          name="dsb")
                            nc.vector.tensor_scalar(
                                out=dsb, in0=dcol,
                                scalar1=expsb[:, ds(2 * g + h, 1)], scalar2=None,
                                op0=ALU.add,
                            )
                            nc.vector.reciprocal(
                                out=rec[g][h][:, ds(4 * qb, 4)], in_=dsb
                            )

                            # numerator rows -> SBUF (bf16) for the wo matmul
                            nc.scalar.copy(
                                out=avT[g][qb][ds(hs, 64), :], in_=av[0:64, :]
                            )

                # ------- P3: output projection, all 8 heads accumulated -------
                for qb in range(NB):
                    for sub in range(4):
                        c = 4 * qb + sub
                        acc = None
                        for g in range(NGRP):
                            wo0 = pp.tile([128, 512], F32, tag="bank", name="wo0")
                            nc.tensor.matmul(
                                wo0,
                                avT[g][qb][0:64, ds(128 * sub, 128)],
                                wob_t[g][0:64, :],
                                start=True, stop=True,
                            )
                            wo1 = pp.tile([128, 512], F32, tag="bank", name="wo1")
                            nc.tensor.matmul(
                                wo1,
                                avT[g][qb][64:128, ds(128 * sub, 128)],
                                wob_t[g][64:128, :],
                                start=True, stop=True,
                            )
                            if acc is None:
                                acc = wp.tile([128, 512], F32, tag="acc0",
                                              bufs=2, name="acc0")
                                nc.scalar.activation(
                                    out=acc, in_=wo0, func=AF.Copy,
                                    scale=rec[g][0][:, ds(c, 1)],
                                )
                            else:
                                nxt = wp.tile([128, 512], F32, tag=f"acc{g}",
                                              bufs=2, name=f"acc{g}")
                                nc.vector.scalar_tensor_tensor(
                                    out=nxt,
                                    in0=wo0,
                                    scalar=rec[g][0][:, ds(c, 1)],
                                    in1=acc,
                                    op0=ALU.mult,
                                    op1=ALU.add,
                                )
                                acc = nxt
                            last = g == NGRP - 1
                            dst = (
                                wp.tile([128, 512], BF16, tag="osb", bufs=3,
                                        name="osb")
                                if last
                                else wp.tile([128, 512], F32, tag=f"accb{g}",
                                             bufs=2, name=f"accb{g}")
                            )
                            nc.vector.scalar_tensor_tensor(
                                out=dst,
                                in0=wo1,
                                scalar=rec[g][1][:, ds(c, 1)],
                                in1=acc,
                                op0=ALU.mult,
                                op1=ALU.add,
                            )
                            acc = dst
                        nc.sync.dma_start(
                            out=outp_d[ds(2048 * b + 128 * c, 128), :], in_=acc)

    nc.compile()
    return nc


def _rope_perm():
    """Row permutation realizing the RoPE half-swap within each 64-dim head."""
    p = np.arange(128)
    base = (p // 64) * 64
    lr = p % 64
    return base + (lr + 32) % 64


def _host_prep(inputs):
    """Build the single-core input map from full inputs."""
    import ml_dtypes

    bf16 = ml_dtypes.bfloat16
    x = np.asarray(inputs["x"], dtype=np.float32)
    wq = np.asarray(inputs["wq"], dtype=np.float32)
    wk = np.asarray(inputs["wk"], dtype=np.float32)
    wv = np.asarray(inputs["wv"], dtype=np.float32)
    wo = np.asarray(inputs["wo"], dtype=np.float32)
    wk_c = np.asarray(inputs["wk_c"], dtype=np.float32)
    wv_c = np.asarray(inputs["wv_c"], dtype=np.float32)
    gate_logits = np.asarray(inputs["gate_logits"], dtype=np.float32)
    sink_logit = np.asarray(inputs["sink_logit"], dtype=np.float32)

    g = np.exp(gate_logits - gate_logits.max())
    g = (g / g.sum()).astype(np.float32)

    perm = _rope_perm()

    blocks = []
    for grp in range(NGRP):
        sl = slice(128 * grp, 128 * (grp + 1))
        wq_s = wq[sl, :]
        wk_s = wk[sl, :]
        blocks += [
            wq_s.T,
            wq_s[perm, :].T,
            wk_s.T,
            wk_s[perm, :].T,
            wv[sl, :].T,
            wk_c[sl, :].T,
            wv_c[sl, :].T,
        ]
    wpk = np.concatenate(blocks, axis=1).astype(bf16)  # [D, 3584]

    misc = np.zeros((1, 16), np.float32)
    misc[0, 0:RATIO] = g
    misc[0, RATIO:RATIO + H] = np.exp(sink_logit[:, 0])

    wob = np.ascontiguousarray(wo.T).astype(bf16)

    xstack = np.concatenate([x[0].T, x[1].T], axis=0)  # [2*D, L]
    in_maps = [
        {
            "xbf": np.ascontiguousarray(xstack).astype(bf16),
            "wpk": np.ascontiguousarray(wpk),
            "wob": wob,
            "misc": misc,
        }
    ]
    return in_maps


def kernel(**inputs) -> np.ndarray:
    from concourse.bass_utils import run_bass_kernel_spmd

    if "nc" not in _CACHE:
        _CACHE["nc"] = _build_nc()
    nc = _CACHE["nc"]

    in_maps = _host_prep(inputs)
    res = run_bass_kernel_spmd(nc, in_maps, core_ids=list(range(NCORES)))
    out = res.results[0]["outp"].astype(np.float32).reshape(B, L, D)
    return out
